# revision 10
# baseline (speedup 1.0000x reference)
"""Multi-head causal self-attention on 8 Trainium2 NeuronCores.

Sharding: core c = 2*b + g handles batch b and head-group g (8 of 16 heads).
  - QKV projections column-parallel (each core computes its 512 out-channels)
  - attention fully local per (batch, head)
  - output projection row-parallel; the two head-group partials of each batch
    are summed on the host during the gather (plus the bias).

All matmuls run in float32r (TF32-like, full PE rate at free-dim >= 256).
Layout choices keep everything transpose-free:
  - qT/kT stored [out_ch, T] so score blocks are computed directly as
    scoresT[tk, tq] = kT_h[:, chunk].T @ qT_h[:, block]   (K = head_dim = 64)
  - v stored natural [T, out_ch] with a ones-column appended per head, so the
    av matmul outT[d+1, tq] = v'_chunk.T @ exp_chunk accumulates the softmax
    denominator (row 64) for free
  - softmax normalization: reciprocal of the denominator row, broadcast across
    64 partitions with a K=1 matmul, then one elementwise multiply
  - causal masking: blocks strictly above the diagonal are skipped entirely;
    the 4 diagonal-crossing chunks per t-block are masked with static 0/1
    tiles after exp

Scheduling: K/V projections for the whole sequence run first (their outputs
are the "history" every later block needs). Then one fused loop over t-blocks
j computes qT(j) just-in-time, attention for all heads at block j, and the
output projection of block j — so ACT (exp) work overlaps PE matmul work
across stages. Score matmuls for even/odd heads are emitted adjacently at
base partitions 0/64 so the PE can run them concurrently in distinct
row-groups; exp runs over chunk-pairs [128, 1024] to amortize ACT overhead.
qT/attnT live as block-local double-buffered tiles to fit SBUF.
"""

import sys

sys.path.insert(0, "/opt/trn_rl_repo")

import numpy as np

import concourse.bass as bass  # noqa: E402
import concourse.mybir as mybir  # noqa: E402
import concourse.tile as tile  # noqa: E402
from concourse import bacc, bass_utils  # noqa: E402

F32 = mybir.dt.float32
F32R = mybir.dt.float32r

B, T, C = 4, 2048, 1024
H_TOT, D = 16, 64
G = 2  # head groups (tensor parallel)
H = H_TOT // G  # heads per core
CO = H * D  # local out-channels (512)
N_CORES = 8
TB = 512  # t block (matmul free dim)
NT = T // TB  # 4 t blocks
NKC = C // 128  # 8 contraction chunks over C
NTC = T // 128  # 16 t chunks of 128
SCALE = 1.0 / np.sqrt(D)

_CACHE: dict = {}


def _build(reps: int = 1):
    nc = bacc.Bacc("TRN2", target_bir_lowering=False, debug=False)
    xT = nc.dram_tensor("xT", [C, T], F32, kind="ExternalInput").ap()
    wq = nc.dram_tensor("wq", [C, CO], F32, kind="ExternalInput").ap()
    wk = nc.dram_tensor("wk", [C, CO], F32, kind="ExternalInput").ap()
    wv = nc.dram_tensor("wv", [C, CO], F32, kind="ExternalInput").ap()
    wp = nc.dram_tensor("wp", [CO, C], F32, kind="ExternalInput").ap()
    msk = nc.dram_tensor("masks", [2, 128, 2 * TB], F32, kind="ExternalInput").ap()
    yT = nc.dram_tensor("yT", [C, T], F32, kind="ExternalOutput").ap()

    with tile.TileContext(nc) as tc:
      for _rep in range(reps):
        with tc.tile_pool(name="persist", bufs=1) as persist, \
             tc.tile_pool(name="wqp", bufs=1) as wqp, \
             tc.tile_pool(name="pacc", bufs=2, space="PSUM") as pacc:
            kT = [persist.tile([128, T], F32R, name=f"kT{m}") for m in range(4)]
            vv = [persist.tile([128, H * 65], F32R, name=f"vv{i}") for i in range(NTC)]
            ones_f32 = persist.tile([128, 1], F32, name="ones_f32")
            nc.vector.memset(ones_f32[:], 1.0)
            ones1r = persist.tile([1, 64], F32R, name="ones1r")
            nc.vector.tensor_copy(ones1r[:], ones_f32[0:1, 0:1].to_broadcast([1, 64]))
            mask_t = [
                persist.tile([128, 2 * TB], F32R, name=f"mask{r}") for r in range(2)
            ]
            wq_t = [wqp.tile([128, CO], F32R, name=f"wq{k}") for k in range(NKC)]
            wp_t = [wqp.tile([128, C], F32R, name=f"wp{g}") for g in range(4)]

            def emit_deferred_loads():
                # issued after the K/V-phase loads so the first matmuls
                # aren't starved behind weights that are only needed later
                for r in range(2):
                    nc.sync.dma_start(mask_t[r][:], msk[r].bitcast(F32R))
                for k in range(NKC):
                    nc.sync.dma_start(
                        wq_t[k][:], wq[k * 128 : (k + 1) * 128, :].bitcast(F32R)
                    )
                for g in range(4):
                    nc.sync.dma_start(
                        wp_t[g][:], wp[g * 128 : (g + 1) * 128, :].bitcast(F32R)
                    )
            for i in range(NTC):
                v3 = vv[i].rearrange("p (h e) -> p h e", e=65)
                nc.vector.tensor_copy(
                    v3[:, :, 64:65],
                    ones_f32[:, 0:1, None].to_broadcast([128, H, 1]),
                )

            # ---- K/V projections for the whole sequence ----
            with tc.tile_pool(name="wkv", bufs=1) as wkv, \
                 tc.tile_pool(name="kvx", bufs=2) as kvx:
                wk_t = [wkv.tile([128, CO], F32R, name=f"wk{k}") for k in range(NKC)]
                wv_t = [wkv.tile([128, CO], F32R, name=f"wv{k}") for k in range(NKC)]
                for k in range(NKC):
                    sl = slice(k * 128, (k + 1) * 128)
                    nc.sync.dma_start(wk_t[k][:], wk[sl, :].bitcast(F32R))
                    nc.sync.dma_start(wv_t[k][:], wv[sl, :].bitcast(F32R))
                for j in range(NT):
                    jsl = slice(j * TB, (j + 1) * TB)
                    x_t = [
                        kvx.tile([128, TB], F32R, name=f"kx{k}", tag=f"kx{k}")
                        for k in range(NKC)
                    ]
                    for k in range(NKC):
                        nc.sync.dma_start(
                            x_t[k][:],
                            xT[k * 128 : (k + 1) * 128, jsl].bitcast(F32R),
                        )
                    if j == 0:
                        emit_deferred_loads()
                    for m in range(4):
                        ps = pacc.tile([128, TB], F32, name="kps", tag="acc")
                        for k in range(NKC):
                            nc.tensor.matmul(
                                ps[:],
                                wk_t[k][:, m * 128 : (m + 1) * 128],
                                x_t[k][:],
                                start=(k == 0),
                                stop=(k == NKC - 1),
                            )
                        nc.vector.tensor_copy(kT[m][:, jsl], ps[:])
                    for tsub in range(4):
                        i = 4 * j + tsub
                        ps = pacc.tile([128, CO], F32, name="vps", tag="acc")
                        for k in range(NKC):
                            nc.tensor.matmul(
                                ps[:],
                                x_t[k][:, tsub * 128 : (tsub + 1) * 128],
                                wv_t[k][:],
                                start=(k == 0),
                                stop=(k == NKC - 1),
                            )
                        v3 = vv[i].rearrange("p (h e) -> p h e", e=65)
                        nc.vector.tensor_copy(
                            v3[:, :, 0:64], ps.rearrange("p (h d) -> p h d", d=64)
                        )

            # ---- fused main loop: qT(j) -> attention(j) -> projection(j) ----
            with tc.tile_pool(name="qblk", bufs=2) as qblk, \
                 tc.tile_pool(name="ablk", bufs=2) as ablk, \
                 tc.tile_pool(name="qx", bufs=2) as qx, \
                 tc.tile_pool(name="bwork", bufs=3) as bwork, \
                 tc.tile_pool(name="brec", bufs=5) as brec, \
                 tc.tile_pool(name="ypool", bufs=2) as ypool, \
                 tc.tile_pool(name="pwide", bufs=1, space="PSUM") as pwide, \
                 tc.tile_pool(name="pout", bufs=1, space="PSUM") as pout:
                def emit_qT(j):
                    jsl = slice(j * TB, (j + 1) * TB)
                    x_t = [
                        qx.tile([128, TB], F32R, name=f"qx{k}", tag=f"qx{k}")
                        for k in range(NKC)
                    ]
                    for k in range(NKC):
                        nc.sync.dma_start(
                            x_t[k][:],
                            xT[k * 128 : (k + 1) * 128, jsl].bitcast(F32R),
                        )
                    qTb = [
                        qblk.tile([128, TB], F32R, name=f"qT{m}", tag=f"qT{m}")
                        for m in range(4)
                    ]
                    for m in range(4):
                        ps = pacc.tile([128, TB], F32, name="qps", tag="acc")
                        for k in range(NKC):
                            nc.tensor.matmul(
                                ps[:],
                                wq_t[k][:, m * 128 : (m + 1) * 128],
                                x_t[k][:],
                                start=(k == 0),
                                stop=(k == NKC - 1),
                            )
                        nc.vector.tensor_copy(qTb[m][:], ps[:])
                    return qTb

                qTb = emit_qT(0)
                for j in range(NT):
                    jsl = slice(j * TB, (j + 1) * TB)
                    # attention for all heads at block j
                    aTb = [
                        ablk.tile([128, TB], F32R, name=f"aT{m}", tag=f"aT{m}")
                        for m in range(4)
                    ]
                    npairs = 2 * j + 2  # tk chunk-pairs (256 wide)
                    pending = []

                    def flush_tail(item):
                        f_hp, f_base, f_raw, f_rec = item
                        b_ps = pacc.tile([64, TB], F32, name="b_ps", tag="acc")
                        nc.tensor.matmul(
                            b_ps[:], ones1r[:], f_rec[:], start=True, stop=True
                        )
                        nc.vector.tensor_mul(
                            out=aTb[f_hp][f_base : f_base + 64, :],
                            in0=f_raw[0:64, :],
                            in1=b_ps[:],
                        )

                    for hp in range(4):
                        o_ps = [
                            pout.tile([65, TB], F32, name="o_ps", tag=f"o{sub}")
                            for sub in range(2)
                        ]
                        for ip in range(npairs):
                            s_ps = []
                            for sub in range(2):
                                base = 64 * sub
                                sp = pwide.tile(
                                    [128, 2 * TB], F32, name="s_ps", tag=f"s{sub}"
                                )
                                s_ps.append(sp)
                                for half in range(2):
                                    i = 2 * ip + half
                                    nc.tensor.matmul(
                                        sp[:, half * TB : (half + 1) * TB],
                                        kT[hp][
                                            base : base + 64, i * 128 : (i + 1) * 128
                                        ],
                                        qTb[hp][base : base + 64, :],
                                        start=True,
                                        stop=True,
                                    )
                            for sub in range(2):
                                h = 2 * hp + sub
                                e_t = bwork.tile(
                                    [128, 2 * TB], F32R, name="e_t", tag="e_t"
                                )
                                nc.scalar.activation(
                                    e_t[:],
                                    s_ps[sub][:],
                                    mybir.ActivationFunctionType.Exp,
                                    scale=float(SCALE),
                                )
                                if ip >= 2 * j:  # diagonal chunk-pair: mask
                                    nc.vector.tensor_mul(
                                        out=e_t[:],
                                        in0=e_t[:],
                                        in1=mask_t[ip - 2 * j][:],
                                    )
                                for half in range(2):
                                    i = 2 * ip + half
                                    nc.tensor.matmul(
                                        o_ps[sub][:],
                                        vv[i][:, 65 * h : 65 * h + 65],
                                        e_t[:, half * TB : (half + 1) * TB],
                                        start=(ip == 0 and half == 0),
                                        stop=(ip == npairs - 1 and half == 1),
                                    )
                        for sub in range(2):
                            base = 64 * sub
                            # free the PSUM bank fast: raw attention out + its
                            # denominator row go to SBUF in one copy, and the
                            # reciprocal runs now; the PE broadcast + final
                            # multiply are deferred one head-pair so the PE
                            # never waits on the DVE chain
                            raw = brec.tile([65, TB], F32R, name="raw", tag="raw")
                            nc.vector.tensor_copy(raw[:], o_ps[sub][:])
                            rec = brec.tile([1, TB], F32R, name="rec", tag="rec")
                            with nc.allow_low_precision(
                                reason="f32r reciprocal: rounding only, ~1e-4"
                            ):
                                nc.vector.reciprocal(rec[:], raw[64:65, :])
                            pending.append((hp, base, raw, rec))
                        while len(pending) > 2:
                            flush_tail(pending.pop(0))

                    # qT for the next block before the projection, so the
                    # PE has work while the last head's softmax tail drains
                    if j + 1 < NT:
                        qTb = emit_qT(j + 1)
                    while pending:
                        flush_tail(pending.pop(0))

                    # output projection for block j
                    for mm in range(8):
                        y_ps = pacc.tile([128, TB], F32, name="y_ps", tag="acc")
                        for g in range(4):
                            nc.tensor.matmul(
                                y_ps[:],
                                wp_t[g][:, mm * 128 : (mm + 1) * 128],
                                aTb[g][:],
                                start=(g == 0),
                                stop=(g == 3),
                            )
                        y_t = ypool.tile([128, TB], F32, name="y_t", tag="y_t")
                        nc.vector.tensor_copy(y_t[:], y_ps[:])
                        nc.sync.dma_start(yT[mm * 128 : (mm + 1) * 128, jsl], y_t[:])

    nc.compile()
    return nc


def _masks() -> np.ndarray:
    p = np.arange(128)[:, None]
    f = np.arange(TB)[None, :]
    m4 = [(f >= 128 * r + p).astype(np.float32) for r in range(4)]
    return np.stack(
        [np.concatenate([m4[0], m4[1]], axis=1),
         np.concatenate([m4[2], m4[3]], axis=1)],
        axis=0,
    )


def build_in_maps(x, Wk, Wq, Wv, Wp):
    x = np.asarray(x, dtype=np.float32)
    Wk = np.asarray(Wk, dtype=np.float32)
    Wq = np.asarray(Wq, dtype=np.float32)
    Wv = np.asarray(Wv, dtype=np.float32)
    Wp = np.asarray(Wp, dtype=np.float32)
    masks = _masks()
    in_maps = []
    for b in range(B):
        xTb = np.ascontiguousarray(x[b].T)
        for g in range(G):
            sl = slice(g * CO, (g + 1) * CO)
            in_maps.append(
                {
                    "xT": xTb,
                    "wq": np.ascontiguousarray(Wq[sl, :].T),
                    "wk": np.ascontiguousarray(Wk[sl, :].T),
                    "wv": np.ascontiguousarray(Wv[sl, :].T),
                    "wp": np.ascontiguousarray(Wp[:, sl].T),
                    "masks": masks,
                }
            )
    return in_maps


def combine(results, bp):
    bp = np.asarray(bp, dtype=np.float32)
    y = np.empty((B, T, C), np.float32)
    for b in range(B):
        acc = results[2 * b]["yT"] + results[2 * b + 1]["yT"]
        y[b] = acc.T + bp[None, :]
    return y


def get_nc(reps: int = 1):
    key = f"nc{reps}"
    if key not in _CACHE:
        _CACHE[key] = _build(reps)
    return _CACHE[key]


def kernel(x, Wk, Wq, Wv, Wp, bp):
    nc = get_nc()
    in_maps = build_in_maps(x, Wk, Wq, Wv, Wp)
    res = bass_utils.run_bass_kernel_spmd(nc, in_maps, list(range(N_CORES)))
    return combine(res.results, bp)


# revision 12
# speedup vs baseline: 182.6045x; 182.6045x over previous
"""Multi-head causal self-attention on 8 Trainium2 NeuronCores.

Sharding: core c = 2*b + g handles batch b and head-group g (8 of 16 heads).
  - QKV projections column-parallel (each core computes its 512 out-channels)
  - attention fully local per (batch, head)
  - output projection row-parallel; the two head-group partials of each batch
    are summed on the host during the gather (plus the bias).

All matmuls run in float32r (TF32-like, full PE rate at free-dim >= 256).
Layout choices keep everything transpose-free:
  - qT/kT stored [out_ch, T] so score blocks are computed directly as
    scoresT[tk, tq] = kT_h[:, chunk].T @ qT_h[:, block]   (K = head_dim = 64)
  - v stored natural [T, out_ch] with a ones-column appended per head, so the
    av matmul outT[d+1, tq] = v'_chunk.T @ exp_chunk accumulates the softmax
    denominator (row 64) for free
  - softmax normalization: reciprocal of the denominator row, broadcast across
    64 partitions with a K=1 matmul, then one elementwise multiply
  - causal masking: blocks strictly above the diagonal are skipped entirely;
    the 4 diagonal-crossing chunks per t-block are masked with static 0/1
    tiles after exp

Scheduling: K/V projections for the whole sequence run first (their outputs
are the "history" every later block needs). Then one fused loop over t-blocks
j computes qT(j) just-in-time, attention for all heads at block j, and the
output projection of block j — so ACT (exp) work overlaps PE matmul work
across stages. Score matmuls for even/odd heads are emitted adjacently at
base partitions 0/64 so the PE can run them concurrently in distinct
row-groups; exp runs over chunk-pairs [128, 1024] to amortize ACT overhead.
qT/attnT live as block-local double-buffered tiles to fit SBUF.
"""

import sys

sys.path.insert(0, "/opt/trn_rl_repo")

import numpy as np

import concourse.bass as bass  # noqa: E402
import concourse.mybir as mybir  # noqa: E402
import concourse.tile as tile  # noqa: E402
from concourse import bacc, bass_utils  # noqa: E402

F32 = mybir.dt.float32
F32R = mybir.dt.float32r

B, T, C = 4, 2048, 1024
H_TOT, D = 16, 64
G = 2  # head groups (tensor parallel)
H = H_TOT // G  # heads per core
CO = H * D  # local out-channels (512)
N_CORES = 8
TB = 512  # t block (matmul free dim)
NT = T // TB  # 4 t blocks
NKC = C // 128  # 8 contraction chunks over C
NTC = T // 128  # 16 t chunks of 128
SCALE = 1.0 / np.sqrt(D)

_CACHE: dict = {}


def _build(reps: int = 1):
    nc = bacc.Bacc("TRN2", target_bir_lowering=False, debug=False)
    xT = nc.dram_tensor("xT", [C, T], F32, kind="ExternalInput").ap()
    wq = nc.dram_tensor("wq", [C, CO], F32, kind="ExternalInput").ap()
    wk = nc.dram_tensor("wk", [C, CO], F32, kind="ExternalInput").ap()
    wv = nc.dram_tensor("wv", [C, CO], F32, kind="ExternalInput").ap()
    wp = nc.dram_tensor("wp", [CO, C], F32, kind="ExternalInput").ap()
    msk = nc.dram_tensor("masks", [2, 128, 2 * TB], F32, kind="ExternalInput").ap()
    yT = nc.dram_tensor("yT", [C, T], F32, kind="ExternalOutput").ap()

    with tile.TileContext(nc) as tc:
      for _rep in range(reps):
        with tc.tile_pool(name="persist", bufs=1) as persist, \
             tc.tile_pool(name="wqp", bufs=1) as wqp, \
             tc.tile_pool(name="pacc", bufs=2, space="PSUM") as pacc:
            qT = [persist.tile([128, T], F32R, name=f"qT{m}") for m in range(4)]
            kT = [persist.tile([128, T], F32R, name=f"kT{m}") for m in range(4)]
            vv = [persist.tile([128, H * 65], F32R, name=f"vv{i}") for i in range(NTC)]
            ones_f32 = persist.tile([128, 1], F32, name="ones_f32")
            nc.vector.memset(ones_f32[:], 1.0)
            ones1r = persist.tile([1, 64], F32R, name="ones1r")
            nc.vector.tensor_copy(ones1r[:], ones_f32[0:1, 0:1].to_broadcast([1, 64]))
            mask_t = [
                persist.tile([128, 2 * TB], F32R, name=f"mask{r}") for r in range(2)
            ]
            wp_t = [wqp.tile([128, C], F32R, name=f"wp{g}") for g in range(4)]

            # ---- phase A: all three projections, block by block ----
            with tc.tile_pool(name="wkv", bufs=1) as wkv, \
                 tc.tile_pool(name="kvx", bufs=2) as kvx:
                wq_t = [wkv.tile([128, CO], F32R, name=f"wq{k}") for k in range(NKC)]
                wk_t = [wkv.tile([128, CO], F32R, name=f"wk{k}") for k in range(NKC)]
                wv_t = [wkv.tile([128, CO], F32R, name=f"wv{k}") for k in range(NKC)]
                # x(0) first so the opening matmuls aren't starved behind
                # 6 MB of weights; then wk (first consumer), then the rest
                x0_t = [
                    kvx.tile([128, TB], F32R, name=f"kx{k}", tag=f"kx{k}")
                    for k in range(NKC)
                ]
                for k in range(NKC):
                    nc.sync.dma_start(
                        x0_t[k][:], xT[k * 128 : (k + 1) * 128, 0:TB].bitcast(F32R)
                    )
                for k in range(NKC):
                    nc.sync.dma_start(
                        wk_t[k][:],
                        wk[k * 128 : (k + 1) * 128, :].bitcast(F32R),
                    )
                for k in range(NKC):
                    sl = slice(k * 128, (k + 1) * 128)
                    nc.sync.dma_start(wv_t[k][:], wv[sl, :].bitcast(F32R))
                    nc.sync.dma_start(wq_t[k][:], wq[sl, :].bitcast(F32R))
                for r in range(2):
                    nc.sync.dma_start(mask_t[r][:], msk[r].bitcast(F32R))
                for i in range(NTC):
                    v3 = vv[i].rearrange("p (h e) -> p h e", e=65)
                    nc.vector.tensor_copy(
                        v3[:, :, 64:65],
                        ones_f32[:, 0:1, None].to_broadcast([128, H, 1]),
                    )
                for j in range(NT):
                    jsl = slice(j * TB, (j + 1) * TB)
                    if j == 0:
                        x_t = x0_t
                    else:
                        x_t = [
                            kvx.tile([128, TB], F32R, name=f"kx{k}", tag=f"kx{k}")
                            for k in range(NKC)
                        ]
                        for k in range(NKC):
                            nc.sync.dma_start(
                                x_t[k][:],
                                xT[k * 128 : (k + 1) * 128, jsl].bitcast(F32R),
                            )
                    for dst, w_t in ((kT, wk_t), (qT, wq_t)):
                        for m in range(4):
                            ps = pacc.tile([128, TB], F32, name="kps", tag="acc")
                            for k in range(NKC):
                                nc.tensor.matmul(
                                    ps[:],
                                    w_t[k][:, m * 128 : (m + 1) * 128],
                                    x_t[k][:],
                                    start=(k == 0),
                                    stop=(k == NKC - 1),
                                )
                            nc.vector.tensor_copy(dst[m][:, jsl], ps[:])
                    for tsub in range(4):
                        i = 4 * j + tsub
                        ps = pacc.tile([128, CO], F32, name="vps", tag="acc")
                        for k in range(NKC):
                            nc.tensor.matmul(
                                ps[:],
                                x_t[k][:, tsub * 128 : (tsub + 1) * 128],
                                wv_t[k][:],
                                start=(k == 0),
                                stop=(k == NKC - 1),
                            )
                        v3 = vv[i].rearrange("p (h e) -> p h e", e=65)
                        nc.vector.tensor_copy(
                            v3[:, :, 0:64], ps.rearrange("p (h d) -> p h d", d=64)
                        )

            # ---- fused main loop: attention(j) -> projection(j) ----
            with tc.tile_pool(name="ablk", bufs=2) as ablk, \
                 tc.tile_pool(name="bwork", bufs=4) as bwork, \
                 tc.tile_pool(name="brec", bufs=5) as brec, \
                 tc.tile_pool(name="ypool", bufs=3) as ypool, \
                 tc.tile_pool(name="pwide", bufs=1, space="PSUM") as pwide, \
                 tc.tile_pool(name="pout", bufs=1, space="PSUM") as pout:
                for g in range(4):
                    nc.sync.dma_start(
                        wp_t[g][:], wp[g * 128 : (g + 1) * 128, :].bitcast(F32R)
                    )
                for j in range(NT):
                    jsl = slice(j * TB, (j + 1) * TB)
                    aTb = [
                        ablk.tile([128, TB], F32R, name=f"aT{m}", tag=f"aT{m}")
                        for m in range(4)
                    ]
                    npairs = 2 * j + 2  # tk chunk-pairs (256 wide)
                    pending = []

                    def flush_tail(item):
                        f_hp, f_base, f_raw, f_rec = item
                        b_ps = pacc.tile([64, TB], F32, name="b_ps", tag="acc")
                        nc.tensor.matmul(
                            b_ps[:], ones1r[:], f_rec[:], start=True, stop=True
                        )
                        nc.vector.tensor_mul(
                            out=aTb[f_hp][f_base : f_base + 64, :],
                            in0=f_raw[0:64, :],
                            in1=b_ps[:],
                        )

                    for hp in range(4):
                        o_ps = [
                            pout.tile([65, TB], F32, name="o_ps", tag=f"o{sub}")
                            for sub in range(2)
                        ]
                        for ip in range(npairs):
                            s_ps = []
                            for sub in range(2):
                                base = 64 * sub
                                sp = pwide.tile(
                                    [128, 2 * TB], F32, name="s_ps", tag=f"s{sub}"
                                )
                                s_ps.append(sp)
                                for half in range(2):
                                    i = 2 * ip + half
                                    nc.tensor.matmul(
                                        sp[:, half * TB : (half + 1) * TB],
                                        kT[hp][
                                            base : base + 64, i * 128 : (i + 1) * 128
                                        ],
                                        qT[hp][base : base + 64, jsl],
                                        start=True,
                                        stop=True,
                                    )
                            for sub in range(2):
                                h = 2 * hp + sub
                                e_t = bwork.tile(
                                    [128, 2 * TB], F32R, name="e_t", tag="e_t"
                                )
                                nc.scalar.activation(
                                    e_t[:],
                                    s_ps[sub][:],
                                    mybir.ActivationFunctionType.Exp,
                                    scale=float(SCALE),
                                )
                                if ip >= 2 * j:  # diagonal chunk-pair: mask
                                    nc.vector.tensor_mul(
                                        out=e_t[:],
                                        in0=e_t[:],
                                        in1=mask_t[ip - 2 * j][:],
                                    )
                                for half in range(2):
                                    i = 2 * ip + half
                                    nc.tensor.matmul(
                                        o_ps[sub][:],
                                        vv[i][:, 65 * h : 65 * h + 65],
                                        e_t[:, half * TB : (half + 1) * TB],
                                        start=(ip == 0 and half == 0),
                                        stop=(ip == npairs - 1 and half == 1),
                                    )
                        for sub in range(2):
                            base = 64 * sub
                            # free the PSUM bank fast; PE broadcast + final
                            # multiply deferred one head-pair so the PE never
                            # waits on the DVE chain
                            raw = brec.tile([65, TB], F32R, name="raw", tag="raw")
                            nc.vector.tensor_copy(raw[:], o_ps[sub][:])
                            rec = brec.tile([1, TB], F32R, name="rec", tag="rec")
                            with nc.allow_low_precision(
                                reason="f32r reciprocal: rounding only, ~1e-4"
                            ):
                                nc.vector.reciprocal(rec[:], raw[64:65, :])
                            pending.append((hp, base, raw, rec))
                        while len(pending) > 2:
                            flush_tail(pending.pop(0))

                    while pending:
                        flush_tail(pending.pop(0))

                    # output projection for block j
                    for mm in range(8):
                        y_ps = pacc.tile([128, TB], F32, name="y_ps", tag="acc")
                        for g in range(4):
                            nc.tensor.matmul(
                                y_ps[:],
                                wp_t[g][:, mm * 128 : (mm + 1) * 128],
                                aTb[g][:],
                                start=(g == 0),
                                stop=(g == 3),
                            )
                        y_t = ypool.tile([128, TB], F32, name="y_t", tag="y_t")
                        nc.vector.tensor_copy(y_t[:], y_ps[:])
                        nc.sync.dma_start(yT[mm * 128 : (mm + 1) * 128, jsl], y_t[:])

    nc.compile()
    return nc


def _masks() -> np.ndarray:
    p = np.arange(128)[:, None]
    f = np.arange(TB)[None, :]
    m4 = [(f >= 128 * r + p).astype(np.float32) for r in range(4)]
    return np.stack(
        [np.concatenate([m4[0], m4[1]], axis=1),
         np.concatenate([m4[2], m4[3]], axis=1)],
        axis=0,
    )


def build_in_maps(x, Wk, Wq, Wv, Wp):
    x = np.asarray(x, dtype=np.float32)
    Wk = np.asarray(Wk, dtype=np.float32)
    Wq = np.asarray(Wq, dtype=np.float32)
    Wv = np.asarray(Wv, dtype=np.float32)
    Wp = np.asarray(Wp, dtype=np.float32)
    masks = _masks()
    in_maps = []
    for b in range(B):
        xTb = np.ascontiguousarray(x[b].T)
        for g in range(G):
            sl = slice(g * CO, (g + 1) * CO)
            in_maps.append(
                {
                    "xT": xTb,
                    "wq": np.ascontiguousarray(Wq[sl, :].T),
                    "wk": np.ascontiguousarray(Wk[sl, :].T),
                    "wv": np.ascontiguousarray(Wv[sl, :].T),
                    "wp": np.ascontiguousarray(Wp[:, sl].T),
                    "masks": masks,
                }
            )
    return in_maps


def combine(results, bp):
    bp = np.asarray(bp, dtype=np.float32)
    y = np.empty((B, T, C), np.float32)
    for b in range(B):
        acc = results[2 * b]["yT"] + results[2 * b + 1]["yT"]
        y[b] = acc.T + bp[None, :]
    return y


def get_nc(reps: int = 1):
    key = f"nc{reps}"
    if key not in _CACHE:
        _CACHE[key] = _build(reps)
    return _CACHE[key]


def kernel(x, Wk, Wq, Wv, Wp, bp):
    nc = get_nc()
    in_maps = build_in_maps(x, Wk, Wq, Wv, Wp)
    res = bass_utils.run_bass_kernel_spmd(nc, in_maps, list(range(N_CORES)))
    return combine(res.results, bp)


# revision 18
# speedup vs baseline: 193.0766x; 1.0573x over previous
"""Multi-head causal self-attention on 8 Trainium2 NeuronCores.

Sharding: core c = 2*b + g handles batch b and head-group g (8 of 16 heads).
  - QKV projections column-parallel (each core computes its 512 out-channels)
  - attention fully local per (batch, head)
  - output projection row-parallel; the two head-group partials of each batch
    are summed on the host during the gather (plus the bias).

All matmuls run in float32r (TF32-like, full PE rate at free-dim >= 256).
Layout choices keep everything transpose-free:
  - qT/kT stored [out_ch, T] so score blocks are computed directly as
    scoresT[tk, tq] = kT_h[:, chunk].T @ qT_h[:, block]   (K = head_dim = 64)
  - v stored natural [T, out_ch] with a ones-column appended per head, so the
    av matmul outT[d+1, tq] = v'_chunk.T @ exp_chunk accumulates the softmax
    denominator (row 64) for free
  - softmax normalization: reciprocal of the denominator row, broadcast across
    64 partitions with a K=1 matmul, then one elementwise multiply
  - causal masking: blocks strictly above the diagonal are skipped entirely;
    the 4 diagonal-crossing chunks per t-block are masked with static 0/1
    tiles after exp

Scheduling: K/V projections for the whole sequence run first (their outputs
are the "history" every later block needs). Then one fused loop over t-blocks
j computes qT(j) just-in-time, attention for all heads at block j, and the
output projection of block j — so ACT (exp) work overlaps PE matmul work
across stages. Score matmuls for even/odd heads are emitted adjacently at
base partitions 0/64 so the PE can run them concurrently in distinct
row-groups; exp runs over chunk-pairs [128, 1024] to amortize ACT overhead.
qT/attnT live as block-local double-buffered tiles to fit SBUF.
"""

import sys

sys.path.insert(0, "/opt/trn_rl_repo")

import numpy as np

import concourse.bass as bass  # noqa: E402
import concourse.mybir as mybir  # noqa: E402
import concourse.tile as tile  # noqa: E402
from concourse import bacc, bass_utils  # noqa: E402

F32 = mybir.dt.float32
F32R = mybir.dt.float32r

B, T, C = 4, 2048, 1024
H_TOT, D = 16, 64
G = 2  # head groups (tensor parallel)
H = H_TOT // G  # heads per core
CO = H * D  # local out-channels (512)
N_CORES = 8
TB = 512  # t block (matmul free dim)
NT = T // TB  # 4 t blocks
NKC = C // 128  # 8 contraction chunks over C
NTC = T // 128  # 16 t chunks of 128
SCALE = 1.0 / np.sqrt(D)

_CACHE: dict = {}


def _build(reps: int = 1):
    nc = bacc.Bacc("TRN2", target_bir_lowering=False, debug=False)
    xT = nc.dram_tensor("xT", [C, T], F32, kind="ExternalInput").ap()
    wq = nc.dram_tensor("wq", [C, CO], F32, kind="ExternalInput").ap()
    wk = nc.dram_tensor("wk", [C, CO], F32, kind="ExternalInput").ap()
    wv = nc.dram_tensor("wv", [C, CO], F32, kind="ExternalInput").ap()
    wp = nc.dram_tensor("wp", [CO, C], F32, kind="ExternalInput").ap()
    msk = nc.dram_tensor("masks", [2, 128, 2 * TB], F32, kind="ExternalInput").ap()
    yT = nc.dram_tensor("yT", [C, T], F32, kind="ExternalOutput").ap()

    with tile.TileContext(nc) as tc:
      for _rep in range(reps):
        with tc.tile_pool(name="persist", bufs=1) as persist, \
             tc.tile_pool(name="wqp", bufs=1) as wqp:
            qT = [persist.tile([128, T], F32R, name=f"qT{m}") for m in range(4)]
            kT = [persist.tile([128, T], F32R, name=f"kT{m}") for m in range(4)]
            vv = [persist.tile([128, H * 65], F32R, name=f"vv{i}") for i in range(NTC)]
            ones_f32 = persist.tile([128, 1], F32, name="ones_f32")
            nc.vector.memset(ones_f32[:], 1.0)
            ones1r = persist.tile([1, 64], F32R, name="ones1r")
            nc.vector.tensor_copy(ones1r[:], ones_f32[0:1, 0:1].to_broadcast([1, 64]))
            mask_t = [
                persist.tile([128, 2 * TB], F32R, name=f"mask{r}") for r in range(2)
            ]
            wp_t = [wqp.tile([128, C], F32R, name=f"wp{g}") for g in range(4)]

            # ---- phase A: all three projections, block by block ----
            with tc.tile_pool(name="wkv", bufs=1) as wkv, \
                 tc.tile_pool(name="kvx", bufs=2) as kvx, \
                 tc.tile_pool(name="pacc", bufs=2, space="PSUM") as pacc:
                wq_t = [wkv.tile([128, CO], F32R, name=f"wq{k}") for k in range(NKC)]
                wk_t = [wkv.tile([128, CO], F32R, name=f"wk{k}") for k in range(NKC)]
                wv_t = [wkv.tile([128, CO], F32R, name=f"wv{k}") for k in range(NKC)]
                # x(0) first so the opening matmuls aren't starved behind
                # 6 MB of weights; then wk (first consumer), then the rest
                x0_t = [
                    kvx.tile([128, TB], F32R, name=f"kx{k}", tag=f"kx{k}")
                    for k in range(NKC)
                ]
                for k in range(NKC):
                    nc.sync.dma_start(
                        x0_t[k][:], xT[k * 128 : (k + 1) * 128, 0:TB].bitcast(F32R)
                    )
                for k in range(NKC):
                    nc.sync.dma_start(
                        wk_t[k][:],
                        wk[k * 128 : (k + 1) * 128, :].bitcast(F32R),
                    )
                for k in range(NKC):
                    sl = slice(k * 128, (k + 1) * 128)
                    nc.sync.dma_start(wv_t[k][:], wv[sl, :].bitcast(F32R))
                    nc.sync.dma_start(wq_t[k][:], wq[sl, :].bitcast(F32R))
                for r in range(2):
                    nc.sync.dma_start(mask_t[r][:], msk[r].bitcast(F32R))
                for i in range(NTC):
                    v3 = vv[i].rearrange("p (h e) -> p h e", e=65)
                    nc.vector.tensor_copy(
                        v3[:, :, 64:65],
                        ones_f32[:, 0:1, None].to_broadcast([128, H, 1]),
                    )
                for j in range(NT):
                    jsl = slice(j * TB, (j + 1) * TB)
                    if j == 0:
                        x_t = x0_t
                    else:
                        x_t = [
                            kvx.tile([128, TB], F32R, name=f"kx{k}", tag=f"kx{k}")
                            for k in range(NKC)
                        ]
                        for k in range(NKC):
                            nc.sync.dma_start(
                                x_t[k][:],
                                xT[k * 128 : (k + 1) * 128, jsl].bitcast(F32R),
                            )
                    for dst, w_t in ((kT, wk_t), (qT, wq_t)):
                        for m in range(4):
                            ps = pacc.tile([128, TB], F32, name="kps", tag="acc")
                            for k in range(NKC):
                                nc.tensor.matmul(
                                    ps[:],
                                    w_t[k][:, m * 128 : (m + 1) * 128],
                                    x_t[k][:],
                                    start=(k == 0),
                                    stop=(k == NKC - 1),
                                )
                            nc.vector.tensor_copy(dst[m][:, jsl], ps[:])
                    for tsub in range(4):
                        i = 4 * j + tsub
                        ps = pacc.tile([128, CO], F32, name="vps", tag="acc")
                        for k in range(NKC):
                            nc.tensor.matmul(
                                ps[:],
                                x_t[k][:, tsub * 128 : (tsub + 1) * 128],
                                wv_t[k][:],
                                start=(k == 0),
                                stop=(k == NKC - 1),
                            )
                        v3 = vv[i].rearrange("p (h e) -> p h e", e=65)
                        nc.vector.tensor_copy(
                            v3[:, :, 0:64], ps.rearrange("p (h d) -> p h d", d=64)
                        )

            # ---- fused main loop: attention(j) -> projection(j) ----
            with tc.tile_pool(name="ablk", bufs=2) as ablk, \
                 tc.tile_pool(name="bwork", bufs=4) as bwork, \
                 tc.tile_pool(name="brec", bufs=5) as brec, \
                 tc.tile_pool(name="ypool", bufs=3) as ypool, \
                 tc.tile_pool(name="pun", bufs=1, space="PSUM") as pun:
                for g in range(4):
                    nc.sync.dma_start(
                        wp_t[g][:], wp[g * 128 : (g + 1) * 128, :].bitcast(F32R)
                    )
                for j in range(NT):
                    jsl = slice(j * TB, (j + 1) * TB)
                    aTb = [
                        ablk.tile([128, TB], F32R, name=f"aT{m}", tag=f"aT{m}")
                        for m in range(4)
                    ]
                    npairs = 2 * j + 2  # tk chunk-pairs (256 wide)
                    pending = []

                    def flush_tail(item):
                        f_hp, f_base, f_raw, f_rec = item
                        b_ps = pun.tile([64, TB], F32, name="b_ps", tag="u", bufs=6)
                        nc.tensor.matmul(
                            b_ps[:], ones1r[:], f_rec[:], start=True, stop=True
                        )
                        nc.vector.tensor_mul(
                            out=aTb[f_hp][f_base : f_base + 64, :],
                            in0=f_raw[0:64, :],
                            in1=b_ps[:],
                        )

                    for hp in range(4):
                        o_ps = [
                            pun.tile([65, TB], F32, name="o_ps", tag="o", bufs=2)
                            for sub in range(2)
                        ]
                        av_prev = None
                        for ip in range(npairs):
                            s_ps = []
                            for sub in range(2):
                                base = 64 * sub
                                sp = [
                                    pun.tile([128, TB], F32, name="s_ps",
                                             tag="u", bufs=6)
                                    for _ in range(2)
                                ]
                                s_ps.append(sp)
                                for half in range(2):
                                    i = 2 * ip + half
                                    nc.tensor.matmul(
                                        sp[half][:],
                                        kT[hp][
                                            base : base + 64, i * 128 : (i + 1) * 128
                                        ],
                                        qT[hp][base : base + 64, jsl],
                                        start=True,
                                        stop=True,
                                    )
                            e_ts = []
                            for sub in range(2):
                                h = 2 * hp + sub
                                e_t = bwork.tile(
                                    [128, 2 * TB], F32R, name="e_t", tag="e_t"
                                )
                                for half in range(2):
                                    nc.scalar.activation(
                                        e_t[:, half * TB : (half + 1) * TB],
                                        s_ps[sub][half][:],
                                        mybir.ActivationFunctionType.Exp,
                                        scale=float(SCALE),
                                    )
                                if ip >= 2 * j:  # diagonal chunk-pair: mask
                                    nc.vector.tensor_mul(
                                        out=e_t[:],
                                        in0=e_t[:],
                                        in1=mask_t[ip - 2 * j][:],
                                    )
                                e_ts.append((sub, h, ip, e_t))
                            # av matmuls run one chunk-pair behind their exp
                            # so the in-order PE stream never waits on ACT
                            if av_prev is not None:
                                for (p_sub, p_h, p_ip, p_et) in av_prev:
                                    for half in range(2):
                                        i = 2 * p_ip + half
                                        nc.tensor.matmul(
                                            o_ps[p_sub][:],
                                            vv[i][:, 65 * p_h : 65 * p_h + 65],
                                            p_et[:, half * TB : (half + 1) * TB],
                                            start=(p_ip == 0 and half == 0),
                                            stop=(p_ip == npairs - 1 and half == 1),
                                        )
                            av_prev = e_ts
                        for (p_sub, p_h, p_ip, p_et) in av_prev:
                            for half in range(2):
                                i = 2 * p_ip + half
                                nc.tensor.matmul(
                                    o_ps[p_sub][:],
                                    vv[i][:, 65 * p_h : 65 * p_h + 65],
                                    p_et[:, half * TB : (half + 1) * TB],
                                    start=(p_ip == 0 and half == 0),
                                    stop=(p_ip == npairs - 1 and half == 1),
                                )
                        for sub in range(2):
                            base = 64 * sub
                            # free the PSUM bank fast; PE broadcast + final
                            # multiply deferred one head-pair so the PE never
                            # waits on the DVE chain
                            raw = brec.tile([65, TB], F32R, name="raw", tag="raw")
                            nc.vector.tensor_copy(raw[:], o_ps[sub][:])
                            rec = brec.tile([1, TB], F32R, name="rec", tag="rec")
                            with nc.allow_low_precision(
                                reason="f32r reciprocal: rounding only, ~1e-4"
                            ):
                                nc.vector.reciprocal(rec[:], raw[64:65, :])
                            pending.append((hp, base, raw, rec))
                        while len(pending) > 2:
                            flush_tail(pending.pop(0))

                    while pending:
                        flush_tail(pending.pop(0))

                    # output projection for block j
                    for mm in range(8):
                        y_ps = pun.tile([128, TB], F32, name="y_ps", tag="u", bufs=6)
                        for g in range(4):
                            nc.tensor.matmul(
                                y_ps[:],
                                wp_t[g][:, mm * 128 : (mm + 1) * 128],
                                aTb[g][:],
                                start=(g == 0),
                                stop=(g == 3),
                            )
                        y_t = ypool.tile([128, TB], F32, name="y_t", tag="y_t")
                        nc.vector.tensor_copy(y_t[:], y_ps[:])
                        nc.sync.dma_start(yT[mm * 128 : (mm + 1) * 128, jsl], y_t[:])

    nc.compile()
    return nc


def _masks() -> np.ndarray:
    p = np.arange(128)[:, None]
    f = np.arange(TB)[None, :]
    m4 = [(f >= 128 * r + p).astype(np.float32) for r in range(4)]
    return np.stack(
        [np.concatenate([m4[0], m4[1]], axis=1),
         np.concatenate([m4[2], m4[3]], axis=1)],
        axis=0,
    )


def build_in_maps(x, Wk, Wq, Wv, Wp):
    x = np.asarray(x, dtype=np.float32)
    Wk = np.asarray(Wk, dtype=np.float32)
    Wq = np.asarray(Wq, dtype=np.float32)
    Wv = np.asarray(Wv, dtype=np.float32)
    Wp = np.asarray(Wp, dtype=np.float32)
    masks = _masks()
    in_maps = []
    for b in range(B):
        xTb = np.ascontiguousarray(x[b].T)
        for g in range(G):
            sl = slice(g * CO, (g + 1) * CO)
            in_maps.append(
                {
                    "xT": xTb,
                    "wq": np.ascontiguousarray(Wq[sl, :].T),
                    "wk": np.ascontiguousarray(Wk[sl, :].T),
                    "wv": np.ascontiguousarray(Wv[sl, :].T),
                    "wp": np.ascontiguousarray(Wp[:, sl].T),
                    "masks": masks,
                }
            )
    return in_maps


def combine(results, bp):
    bp = np.asarray(bp, dtype=np.float32)
    y = np.empty((B, T, C), np.float32)
    for b in range(B):
        acc = results[2 * b]["yT"] + results[2 * b + 1]["yT"]
        y[b] = acc.T + bp[None, :]
    return y


def get_nc(reps: int = 1):
    key = f"nc{reps}"
    if key not in _CACHE:
        _CACHE[key] = _build(reps)
    return _CACHE[key]


def kernel(x, Wk, Wq, Wv, Wp, bp):
    nc = get_nc()
    in_maps = build_in_maps(x, Wk, Wq, Wv, Wp)
    res = bass_utils.run_bass_kernel_spmd(nc, in_maps, list(range(N_CORES)))
    return combine(res.results, bp)
